# revision 2
# baseline (speedup 1.0000x reference)
"""Trainium2 Bass kernel for linear causal self-attention (ELU+1 feature map).

Model (per batch b):
    qkv = x @ W_attn + b_attn ; q,k,v split; per-head (H=16, d=64)
    phi = elu(.)+1 applied to q,k
    causal linear attention: y_t = (phi_q_t . KV_t) / (phi_q_t . Ksum_t + eps)
        KV_t = sum_{s<=t} phi_k_s (x) v_s ; Ksum_t = sum_{s<=t} phi_k_s
    out = y @ W_proj + b_proj

Sharding (8 cores): core = 2*b + g  (b in 0..3 batches, g in 0..1 head-groups
of 8 heads).  Each core computes a partial output for its batch over its 8
heads; the host sums the two partials per batch and adds b_proj.

On-chip: chunked linear attention (chunk S=256).  All matmuls run in bf16
(f32 PSUM accumulation; the KV state master, denominators and normalization
stay f32).  q/k are produced feature-major; v is produced token-major
directly (x-tile stationary); phi(k) is re-laid-out token-major via PE
transposes.  ELU+1 is computed exactly as max(min(exp(x+b), 1), (x+b)+1)
with one ScalarE Exp pass and one fused custom DVE op.
"""

from contextlib import ExitStack

import ml_dtypes
import numpy as np

import concourse.bass as bass
import concourse.mybir as mybir
import concourse.tile as tile
from concourse.bass_utils import run_bass_kernel_spmd
from concourse.masks import make_identity

F32 = mybir.dt.float32
BF16 = mybir.dt.bfloat16
AF = mybir.ActivationFunctionType

# Problem shape (hardcoded per harness contract)
B, T, C = 4, 2048, 1024
NH, D = 16, 64          # heads total, head dim
HG = 8                  # heads per core (head-group)
HP = 4                  # head-pairs per core (2 heads stacked on 128 partitions)
FC = HG * D             # 512 features per core per q/k/v
KT = C // 128           # 8 k-tiles of the model dim
SLAB = 512              # tokens per slab
NSLAB = T // SLAB       # 4
S = 256                 # chunk length (2 chunks per slab)
N_CORES = 8


def _split_waits(nc, max_waits=1):
    """This walrus build accepts at most one embedded sync-wait per
    instruction; hoist extras into standalone EventSemaphore instructions."""
    n = 0
    for fn in nc.m.functions:
        for bb in fn.blocks:
            new_insts = []
            for inst in bb.instructions:
                si = inst.sync_info
                if si is not None and si.on_wait and len(si.on_wait) > max_waits:
                    extra = si.on_wait[:-max_waits]
                    keep = si.on_wait[-max_waits:]
                    for w in extra:
                        ev = mybir.InstEventSemaphore(
                            name=f"{inst.name}-wsplit{n}",
                            ins=[], outs=[],
                            engine=inst.engine,
                            sync_info=mybir.SyncInfo(on_wait=[w], on_update=[]),
                        )
                        n += 1
                        new_insts.append(ev)
                    si.on_wait = list(keep)
                new_insts.append(inst)
            bb.instructions = new_insts
    return n


class _Ctx:
    """Holds nc, pools and per-kernel constant tiles."""


def _setup_consts(g: _Ctx, dram):
    """Emit const DMAs in critical-path order: wq + biases + xt slab 0
    first (q compute for slab 0 needs exactly these), then wk/wv, the
    remaining x slabs, and wp (only needed ~60us in at first proj)."""
    nc, consts = g.nc, g.consts
    xt_r = dram["xt"][:].rearrange("(kt p) t -> p kt t", p=128)

    g.wq_sb = consts.tile([128, KT, FC], BF16)
    nc.sync.dma_start(g.wq_sb, dram["wq"][:].rearrange("(kt p) f -> p kt f", p=128))
    # combined scalar biases: [128, 5, HP] = (bq, bq1, bk, bk1, bv)
    g.bias_sb = consts.tile([128, 5, HP], F32)
    nc.sync.dma_start(g.bias_sb, dram["bias"][:])
    g.bq_sb = g.bias_sb[:, 0]
    g.bq1_sb = g.bias_sb[:, 1]
    g.bk_sb = g.bias_sb[:, 2]
    g.bk1_sb = g.bias_sb[:, 3]
    g.bv_sb = g.bias_sb[:, 4]
    # v bias broadcast down partitions: [128, HG, D] f32 (for token-major v)
    g.bvn_sb = consts.tile([128, HG, D], F32)
    nc.sync.dma_start(g.bvn_sb, dram["bvn"][:])

    g.xt_sb = []
    for s in range(NSLAB):
        xt = g.xtp.tile([128, KT, SLAB], BF16, tag=f"xt{s}", name=f"xt{s}")
        g.xt_sb.append(xt)
    nc.sync.dma_start(g.xt_sb[0], xt_r[:, :, 0:SLAB])

    g.wk_sb = consts.tile([128, KT, FC], BF16)
    nc.sync.dma_start(g.wk_sb, dram["wk"][:].rearrange("(kt p) f -> p kt f", p=128))
    g.wv_sb = consts.tile([128, KT, FC], BF16)
    nc.sync.dma_start(g.wv_sb, dram["wv"][:].rearrange("(kt p) f -> p kt f", p=128))

    for s in range(1, NSLAB):
        nc.sync.dma_start(g.xt_sb[s], xt_r[:, :, s * SLAB : (s + 1) * SLAB])

    g.wp_sb = consts.tile([128, HP, C], BF16)
    nc.sync.dma_start(g.wp_sb, dram["wp"][:].rearrange("(fp p) e -> p fp e", p=128))

    # combined causal mask for a [s_k-tile(2), s_q(256)] score block:
    # slot 0 (k-tile 0): keep s_q >= s_k  (tri | all-ones)
    # slot 1 (k-tile 1): left half 0, right half tri
    g.mask = consts.tile([128, 2, S], F32)
    nc.vector.memset(g.mask, 1.0)
    nc.gpsimd.affine_select(
        out=g.mask[:, 0], in_=g.mask[:, 0], compare_op=mybir.AluOpType.is_ge,
        fill=0.0, base=0, pattern=[[1, S]], channel_multiplier=-1,
    )
    nc.gpsimd.affine_select(
        out=g.mask[:, 1], in_=g.mask[:, 1], compare_op=mybir.AluOpType.is_ge,
        fill=0.0, base=-128, pattern=[[1, S]], channel_multiplier=-1,
    )
    g.ident = consts.tile([128, 128], BF16)
    make_identity(nc, g.ident)

    # per-head-pair state master (f32): rows 0:64 head A, 64:128 head B;
    # cols 0:64 = KV[i, j], col 64 = Ksum[i]
    g.kv = []
    for hp in range(HP):
        st = g.statep.tile([128, D + 1], F32, tag=f"kv{hp}", name=f"kv{hp}")
        nc.vector.memset(st, 0.0)
        g.kv.append(st)

    # score PSUM tiles (hoisted; bufs=1 pools).  The masked k-tile-1 matmul
    # only writes the right half; zero the never-written left half once so
    # the mask multiply never reads NaN garbage.
    g.pp = []
    for h in range(2):
        ph = getattr(g, f"ps_p{h}").tile([128, 2, S], F32, tag=f"p{h}", name=f"p{h}")
        nc.vector.memset(ph[:, 1, 0:128], 0.0)
        g.pp.append(ph)


def _feature_major(g: _Ctx, s):
    """q/k in [feature, token] layout per head-pair with ELU+1 applied."""
    nc = g.nc
    xt_sb = g.xt_sb[s]
    qt, kt_ = [], []
    for which, wsb, lst in (("q", g.wq_sb, qt), ("k", g.wk_sb, kt_)):
        for hp in range(HP):
            ps = g.ps_qkv.tile([128, SLAB], F32, tag="qkv", name="qkvps")
            for k in range(KT):
                nc.tensor.matmul(
                    ps,
                    wsb[:, k, hp * 128 : (hp + 1) * 128],
                    xt_sb[:, k, :],
                    start=(k == 0),
                    stop=(k == KT - 1),
                )
            dst = g.qkp.tile([128, SLAB], BF16, tag=f"{which}t{hp}", name=f"{which}t{hp}")
            # elu(y)+1 = max(min(exp(y), 1), y+1) with y = x + b
            bsb = g.bq_sb if which == "q" else g.bk_sb
            b1sb = g.bq1_sb if which == "q" else g.bk1_sb
            e = g.ep.tile([128, SLAB], BF16, tag="e", name="e")
            nc.scalar.activation(
                e, ps, AF.Exp, bias=bsb[:, hp : hp + 1], scale=1.0
            )
            nc.vector.tensor_scalar_min(e, e, 1.0)
            nc.vector.scalar_tensor_tensor(
                dst, ps, b1sb[:, hp : hp + 1], e,
                mybir.AluOpType.add, mybir.AluOpType.max,
            )
            lst.append(dst)
    return qt, kt_


def _naturalize(g: _Ctx, s, kt_):
    """phi(k): PE-transpose into [token, feature] tiles.  v: computed
    token-major directly (x-tile stationary, wv streaming) with the bias
    added by DVE and the ones column (col 64 per head) used to carry
    Ksum / the denominator."""
    nc = g.nc
    xt_sb = g.xt_sb[s]
    knat, vaug = [], []
    for tt in range(SLAB // 128):
        tks = g.ps_qkv.tile([128, HP, 128], BF16, tag="qkv", name="tks")
        for hp in range(HP):
            nc.tensor.transpose(
                tks[:, hp], kt_[hp][:, tt * 128 : (tt + 1) * 128], g.ident
            )
        kn = g.kvp.tile([128, FC], BF16, tag=f"kn{tt}", name=f"kn{tt}")
        nc.scalar.activation(
            kn[:].rearrange("p (a b) -> p a b", a=HP), tks, AF.Copy
        )
        knat.append(kn)

        # v token-major: out[t, f] = sum_c xt[c, t] wv[c, f]
        vps = g.ps_qkv.tile([128, SLAB], F32, tag="qkv", name="vps")
        for k in range(KT):
            nc.tensor.matmul(
                vps,
                xt_sb[:, k, tt * 128 : (tt + 1) * 128],
                g.wv_sb[:, k, :],
                start=(k == 0),
                stop=(k == KT - 1),
            )
        va = g.kvp.tile([128, HG, D + 1], BF16, tag=f"va{tt}", name=f"va{tt}")
        nc.vector.tensor_add(
            va[:, :, 0:D],
            vps[:].rearrange("p (a b) -> p a b", a=HG),
            g.bvn_sb,
        )
        nc.vector.memset(va[:, :, D], 1.0)
        vaug.append(va)
    return knat, vaug


def _chunk(g: _Ctx, c, hp, qt, kt_, knat, vaug, yts):
    """One (chunk, head-pair) step of the chunked linear attention."""
    nc = g.nc
    q0 = c * S
    tt0, tt1 = 2 * c, 2 * c + 1
    qth, kth = qt[hp], kt_[hp]
    # working bf16 copy of the f32 state for the PE
    kvb = g.ynp.tile([128, D + 1], BF16, tag="kvb", name="kvb")
    nc.scalar.activation(kvb, g.kv[hp], AF.Copy)
    # Per head: scores P [s_k-tile(2), s_q] in its own PSUM bank (two
    # matmuls share a row group, so sequential same-bank writes are safe;
    # across heads the row groups differ, hence separate banks).
    pms = []
    for h in range(2):
        r0, r1 = h * D, (h + 1) * D
        ph = g.pp[h]
        nc.tensor.matmul(
            ph[:, 0],
            kth[r0:r1, q0 : q0 + 128],
            qth[r0:r1, q0 : q0 + S],
            tile_position=(h * D, 0),
        )
        # k-tile 1 only scores the causally valid right half of s_q.
        nc.tensor.matmul(
            ph[:, 1, 128:S],
            kth[r0:r1, q0 + 128 : q0 + S],
            qth[r0:r1, q0 + 128 : q0 + S],
            tile_position=(h * D, 0),
        )
        pm = g.pmp.tile([128, 2, S], BF16, tag=f"pm{h}", name=f"pm{h}")
        nc.vector.tensor_mul(pm, ph, g.mask)
        pms.append(pm)
    # y natural [s_q, (u, 65)]: per-head PSUM bank
    ys = []
    for h in range(2):
        r0, r1 = h * D, (h + 1) * D
        hv = hp * 2 + h
        v0 = vaug[tt0][:, hv]
        v1 = vaug[tt1][:, hv]
        pm = pms[h]
        y = getattr(g, f"ps_y{h}").tile(
            [128, 2, D + 1], F32, tag=f"y{h}", name=f"y{h}"
        )
        nc.tensor.matmul(y[:, 0], pm[:, 0, 0:128], v0, start=True, stop=False)
        nc.tensor.matmul(
            y[:, 0],
            qth[r0:r1, q0 : q0 + 128],
            kvb[r0:r1, :],
            start=False, stop=True,
            tile_position=(h * D, 0),
        )
        nc.tensor.matmul(y[:, 1], pm[:, 0, 128:S], v0, start=True, stop=False)
        nc.tensor.matmul(y[:, 1], pm[:, 1, 128:S], v1, start=False, stop=False)
        nc.tensor.matmul(
            y[:, 1],
            qth[r0:r1, q0 + 128 : q0 + S],
            kvb[r0:r1, :],
            start=False, stop=True,
            tile_position=(h * D, 0),
        )
        ys.append(y)
    # state update: KV += Knat^T @ V_aug (heads col-tiled; one bank is fine
    # since col groups write disjoint partition ranges)
    dlt = g.ps_d.tile([128, D + 1], F32, tag="d", name="dlt")
    for h in range(2):
        hv = hp * 2 + h
        nc.tensor.matmul(
            dlt[h * D : (h + 1) * D, :],
            knat[tt0][:, hv * D : (hv + 1) * D],
            vaug[tt0][:, hv],
            start=True, stop=False,
            tile_position=(0, h * D),
        )
        nc.tensor.matmul(
            dlt[h * D : (h + 1) * D, :],
            knat[tt1][:, hv * D : (hv + 1) * D],
            vaug[tt1][:, hv],
            start=False, stop=True,
            tile_position=(0, h * D),
        )
    # normalize y_n = y[..., 0:64] / y[..., 64], transpose into yts
    for h in range(2):
        y = ys[h]
        rec = g.ynp.tile([128, 2], F32, tag="rec", name="rec")
        nc.vector.reciprocal(rec, y[:, :, D])
        yn = g.ynp.tile([128, 2, D], BF16, tag="yn", name="yn")
        nc.vector.tensor_mul(
            yn, y[:, :, 0:D], rec[:, :, None].to_broadcast((128, 2, D))
        )
        # one fused [128,128] transpose: rows (u,d), cols = token%128
        tp = g.ps_t.tile([128, 128], BF16, tag="tp", name="tp")
        nc.tensor.transpose(tp, yn[:].rearrange("p a b -> p (a b)"), g.ident)
        for u in range(2):
            nc.scalar.activation(
                yts[hp][h * D : (h + 1) * D, q0 + u * 128 : q0 + (u + 1) * 128],
                tp[u * D : (u + 1) * D, :],
                AF.Copy,
            )
    # state add (scheduled after the kvb copy above)
    nc.vector.tensor_add(g.kv[hp], g.kv[hp], dlt)


def _proj(g: _Ctx, s, yts, out):
    nc = g.nc
    t0 = s * SLAB
    for tt in range(SLAB // 128):
        for ec in range(2):
            ps = g.ps_qkv.tile([128, SLAB], F32, tag="qkv", name="projps")
            for fp in range(HP):
                nc.tensor.matmul(
                    ps,
                    yts[fp][:, tt * 128 : (tt + 1) * 128],
                    g.wp_sb[:, fp, ec * 512 : (ec + 1) * 512],
                    start=(fp == 0),
                    stop=(fp == HP - 1),
                )
            osb = g.outp.tile([128, 512], BF16, tag="osb", name="osb")
            nc.scalar.activation(osb, ps, AF.Copy)
            nc.sync.dma_start(
                out[t0 + tt * 128 : t0 + (tt + 1) * 128, ec * 512 : (ec + 1) * 512],
                osb,
            )


def build_nc(repeat: int = 1) -> bass.Bass:
    nc = bass.Bass()
    dram = {
        "xt": nc.dram_tensor("xt", [C, T], BF16, kind="ExternalInput"),
        "wq": nc.dram_tensor("wq", [C, FC], BF16, kind="ExternalInput"),
        "wk": nc.dram_tensor("wk", [C, FC], BF16, kind="ExternalInput"),
        "wv": nc.dram_tensor("wv", [C, FC], BF16, kind="ExternalInput"),
        "bias": nc.dram_tensor("bias", [128, 5, HP], F32, kind="ExternalInput"),
        "bvn": nc.dram_tensor("bvn", [128, HG, D], F32, kind="ExternalInput"),
        "wp": nc.dram_tensor("wp", [FC, C], BF16, kind="ExternalInput"),
    }
    out = nc.dram_tensor("out", [T, C], BF16, kind="ExternalOutput")

    with ExitStack() as ctx:
        tc = ctx.enter_context(tile.TileContext(nc))
        g = _Ctx()
        g.nc = nc
        for nm, kw in (
            ("consts", dict(bufs=1)),
            ("xtp", dict(bufs=1)),
            ("qkp", dict(bufs=3)),
            ("kvp", dict(bufs=2)),
            ("ytp", dict(bufs=2)),
            ("ep", dict(bufs=4)),
            ("pmp", dict(bufs=3)),
            ("ynp", dict(bufs=4)),
            ("outp", dict(bufs=3)),
            ("statep", dict(bufs=1)),
            ("ps_qkv", dict(bufs=2, space="PSUM")),
            ("ps_p0", dict(bufs=1, space="PSUM")),
            ("ps_p1", dict(bufs=1, space="PSUM")),
            ("ps_y0", dict(bufs=1, space="PSUM")),
            ("ps_y1", dict(bufs=1, space="PSUM")),
            ("ps_d", dict(bufs=1, space="PSUM")),
            ("ps_t", dict(bufs=1, space="PSUM")),
        ):
            setattr(g, nm, ctx.enter_context(tc.tile_pool(name=nm, **kw)))

        _setup_consts(g, dram)

        for s in range(NSLAB):
            qt, kt_ = _feature_major(g, s)
            knat, vaug = _naturalize(g, s, kt_)
            yts = [
                g.ytp.tile([128, SLAB], BF16, tag=f"yt{hp}", name=f"yt{hp}")
                for hp in range(HP)
            ]
            for c in range(2):
                for hp in range(HP):
                    _chunk(g, c, hp, qt, kt_, knat, vaug, yts)
            _proj(g, s, yts, out)
    _split_waits(nc)
    return nc


_NC_CACHE = None


def _get_nc():
    global _NC_CACHE
    if _NC_CACHE is None:
        _NC_CACHE = build_nc()
    return _NC_CACHE


def make_in_maps(x, W_attn, b_attn, W_proj):
    x = np.asarray(x, dtype=np.float32)
    W_attn = np.asarray(W_attn, dtype=np.float32)
    b_attn = np.asarray(b_attn, dtype=np.float32)
    W_proj = np.asarray(W_proj, dtype=np.float32)
    bf = ml_dtypes.bfloat16
    in_maps = []
    for core in range(N_CORES):
        b, gg = core // 2, core % 2
        qs, ks, vs = gg * FC, C + gg * FC, 2 * C + gg * FC
        bqc = b_attn[qs : qs + FC].reshape(HP, 128).T
        bkc = b_attn[ks : ks + FC].reshape(HP, 128).T
        bvc = b_attn[vs : vs + FC].reshape(HP, 128).T
        bias = np.stack([bqc, bqc + 1.0, bkc, bkc + 1.0, bvc], axis=1)
        bvn = np.broadcast_to(
            b_attn[vs : vs + FC].reshape(1, HG, D), (128, HG, D)
        )
        in_maps.append({
            "xt": np.ascontiguousarray(x[b].T).astype(bf),
            "wq": np.ascontiguousarray(W_attn[:, qs : qs + FC]).astype(bf),
            "wk": np.ascontiguousarray(W_attn[:, ks : ks + FC]).astype(bf),
            "wv": np.ascontiguousarray(W_attn[:, vs : vs + FC]).astype(bf),
            "bias": np.ascontiguousarray(bias),
            "bvn": np.ascontiguousarray(bvn),
            "wp": np.ascontiguousarray(W_proj[gg * FC : (gg + 1) * FC, :]).astype(bf),
        })
    return in_maps


def kernel(x, W_attn, b_attn, W_proj, b_proj, _trace=False):
    nc = _get_nc()
    in_maps = make_in_maps(x, W_attn, b_attn, W_proj)
    try:
        res = run_bass_kernel_spmd(
            nc, in_maps, core_ids=list(range(N_CORES)), trace=_trace
        )
    except ModuleNotFoundError:
        # axon NTFF profiling hook unavailable in this environment
        res = run_bass_kernel_spmd(
            nc, in_maps, core_ids=list(range(N_CORES)), trace=False
        )
    b_proj = np.asarray(b_proj, dtype=np.float32)
    parts = [r["out"].astype(np.float32) for r in res.results]
    out = np.stack(
        [parts[2 * b] + parts[2 * b + 1] + b_proj for b in range(B)]
    ).astype(np.float32)
    kernel.last_results = res
    return out


# revision 8
# speedup vs baseline: 1.2299x; 1.2299x over previous
"""Trainium2 Bass kernel for linear causal self-attention (ELU+1 feature map).

Model (per batch b):
    qkv = x @ W_attn + b_attn ; q,k,v split; per-head (H=16, d=64)
    phi = elu(.)+1 applied to q,k
    causal linear attention: y_t = (phi_q_t . KV_t) / (phi_q_t . Ksum_t + eps)
        KV_t = sum_{s<=t} phi_k_s (x) v_s ; Ksum_t = sum_{s<=t} phi_k_s
    out = y @ W_proj + b_proj

Sharding (8 cores): core = 2*b + g  (b in 0..3 batches, g in 0..1 head-groups
of 8 heads).  Each core computes a partial output for its batch over its 8
heads; the host sums the two partials per batch and adds b_proj.

On-chip: chunked linear attention (chunk S=256).  All matmuls run in bf16
(f32 PSUM accumulation; the KV state master, denominators and normalization
stay f32).  q/k are produced feature-major; v is produced token-major
directly (x-tile stationary); phi(k) is re-laid-out token-major via PE
transposes.  ELU+1 is computed exactly as max(min(exp(x+b), 1), (x+b)+1)
with one ScalarE Exp pass and one fused custom DVE op.
"""

from contextlib import ExitStack

import ml_dtypes
import numpy as np

import concourse.bass as bass
import concourse.mybir as mybir
import concourse.tile as tile
from concourse.bass_utils import run_bass_kernel_spmd
from concourse.masks import make_identity

F32 = mybir.dt.float32
BF16 = mybir.dt.bfloat16
AF = mybir.ActivationFunctionType

# Problem shape (hardcoded per harness contract)
B, T, C = 4, 2048, 1024
NH, D = 16, 64          # heads total, head dim
HG = 8                  # heads per core (head-group)
HP = 4                  # head-pairs per core (2 heads stacked on 128 partitions)
FC = HG * D             # 512 features per core per q/k/v
KT = C // 128           # 8 k-tiles of the model dim
SLAB = 512              # tokens per slab
NSLAB = T // SLAB       # 4
S = 256                 # chunk length (2 chunks per slab)
N_CORES = 8


def _split_waits(nc, max_waits=1):
    """This walrus build accepts at most one embedded sync-wait per
    instruction; hoist extras into standalone EventSemaphore instructions."""
    n = 0
    for fn in nc.m.functions:
        for bb in fn.blocks:
            new_insts = []
            for inst in bb.instructions:
                si = inst.sync_info
                if si is not None and si.on_wait and len(si.on_wait) > max_waits:
                    extra = si.on_wait[:-max_waits]
                    keep = si.on_wait[-max_waits:]
                    for w in extra:
                        ev = mybir.InstEventSemaphore(
                            name=f"{inst.name}-wsplit{n}",
                            ins=[], outs=[],
                            engine=inst.engine,
                            sync_info=mybir.SyncInfo(on_wait=[w], on_update=[]),
                        )
                        n += 1
                        new_insts.append(ev)
                    si.on_wait = list(keep)
                new_insts.append(inst)
            bb.instructions = new_insts
    return n


class _Ctx:
    """Holds nc, pools and per-kernel constant tiles."""


def _setup_consts(g: _Ctx, dram):
    """Emit const DMAs in critical-path order: wq + biases + xt slab 0
    first (q compute for slab 0 needs exactly these), then wk/wv, the
    remaining x slabs, and wp (only needed ~60us in at first proj)."""
    nc, consts = g.nc, g.consts
    xt_r = dram["xt"][:].rearrange("(kt p) t -> p kt t", p=128)

    g.wq_sb = consts.tile([128, KT, FC], BF16)
    nc.sync.dma_start(g.wq_sb, dram["wq"][:].rearrange("(kt p) f -> p kt f", p=128))
    # combined scalar biases: [128, 5, HP] = (bq, bq1, bk, bk1, bv)
    g.bias_sb = consts.tile([128, 5, HP], F32)
    nc.sync.dma_start(g.bias_sb, dram["bias"][:])
    g.bq_sb = g.bias_sb[:, 0]
    g.bq1_sb = g.bias_sb[:, 1]
    g.bk_sb = g.bias_sb[:, 2]
    g.bk1_sb = g.bias_sb[:, 3]
    g.bv_sb = g.bias_sb[:, 4]

    g.xt_sb = []
    for s in range(NSLAB):
        xt = g.xtp.tile([128, KT, SLAB], BF16, tag=f"xt{s}", name=f"xt{s}")
        g.xt_sb.append(xt)
    nc.sync.dma_start(g.xt_sb[0], xt_r[:, :, 0:SLAB])

    g.wk_sb = consts.tile([128, KT, FC], BF16)
    nc.sync.dma_start(g.wk_sb, dram["wk"][:].rearrange("(kt p) f -> p kt f", p=128))
    g.wv_sb = consts.tile([128, KT, FC], BF16)
    nc.sync.dma_start(g.wv_sb, dram["wv"][:].rearrange("(kt p) f -> p kt f", p=128))

    for s in range(1, NSLAB):
        nc.sync.dma_start(g.xt_sb[s], xt_r[:, :, s * SLAB : (s + 1) * SLAB])

    g.wp_sb = consts.tile([128, HP, C], BF16)
    nc.sync.dma_start(g.wp_sb, dram["wp"][:].rearrange("(fp p) e -> p fp e", p=128))

    # combined causal mask for a [s_k-tile(2), s_q(256)] score block:
    # slot 0 (k-tile 0): keep s_q >= s_k  (tri | all-ones)
    # slot 1 (k-tile 1): left half 0, right half tri
    g.mask = consts.tile([128, 2, S], F32)
    nc.vector.memset(g.mask, 1.0)
    nc.gpsimd.affine_select(
        out=g.mask[:, 0], in_=g.mask[:, 0], compare_op=mybir.AluOpType.is_ge,
        fill=0.0, base=0, pattern=[[1, S]], channel_multiplier=-1,
    )
    nc.gpsimd.affine_select(
        out=g.mask[:, 1], in_=g.mask[:, 1], compare_op=mybir.AluOpType.is_ge,
        fill=0.0, base=-128, pattern=[[1, S]], channel_multiplier=-1,
    )
    g.ident = consts.tile([128, 128], BF16)
    make_identity(nc, g.ident)

    # per-head-pair state master (f32): rows 0:64 head A, 64:128 head B;
    # cols 0:64 = KV[i, j], col 64 = Ksum[i]
    g.kv = []
    for hp in range(HP):
        st = g.statep.tile([128, D + 1], F32, tag=f"kv{hp}", name=f"kv{hp}")
        nc.vector.memset(st, 0.0)
        g.kv.append(st)

    # score PSUM tiles (hoisted; bufs=1 pools).  The masked k-tile-1 matmul
    # only writes the right half; zero the never-written left half once so
    # the mask multiply never reads NaN garbage.
    g.pp = []
    for h in range(2):
        ph = getattr(g, f"ps_p{h}").tile([128, 2, S], F32, tag=f"p{h}", name=f"p{h}")
        nc.vector.memset(ph[:, 1, 0:128], 0.0)
        g.pp.append(ph)


def _feature_major(g: _Ctx, s):
    """q/k/v in [feature, token] layout per head-pair; ELU+1 on q,k; bias on v."""
    nc = g.nc
    xt_sb = g.xt_sb[s]
    qt, kt_, vt = [], [], []
    for which, wsb, lst in (("q", g.wq_sb, qt), ("k", g.wk_sb, kt_), ("v", g.wv_sb, vt)):
        for hp in range(HP):
            ps = g.ps_qkv.tile([128, SLAB], F32, tag="qkv", name="qkvps")
            for k in range(KT):
                nc.tensor.matmul(
                    ps,
                    wsb[:, k, hp * 128 : (hp + 1) * 128],
                    xt_sb[:, k, :],
                    start=(k == 0),
                    stop=(k == KT - 1),
                )
            dst = g.qkp.tile([128, SLAB], BF16, tag=f"{which}t{hp}", name=f"{which}t{hp}")
            if which == "v":
                nc.scalar.activation(
                    dst, ps, AF.Identity, bias=g.bv_sb[:, hp : hp + 1], scale=1.0
                )
            else:
                # elu(y)+1 = max(min(exp(y), 1), y+1) with y = x + b
                bsb = g.bq_sb if which == "q" else g.bk_sb
                b1sb = g.bq1_sb if which == "q" else g.bk1_sb
                e = g.ep.tile([128, SLAB], BF16, tag="e", name="e")
                nc.scalar.activation(
                    e, ps, AF.Exp, bias=bsb[:, hp : hp + 1], scale=1.0
                )
                nc.vector.tensor_scalar_min(e, e, 1.0)
                nc.vector.scalar_tensor_tensor(
                    dst, ps, b1sb[:, hp : hp + 1], e,
                    mybir.AluOpType.add, mybir.AluOpType.max,
                )
            lst.append(dst)
    return qt, kt_, vt


def _naturalize(g: _Ctx, kt_, vt):
    """PE-transpose phi(k) and v into [token, feature] tiles; v gets the
    ones column (col 64 per head) used to carry Ksum / the denominator."""
    nc = g.nc
    knat, vaug = [], []
    for tt in range(SLAB // 128):
        tks = g.ps_qkv.tile([128, HP, 128], BF16, tag="qkv", name="tks")
        for hp in range(HP):
            nc.tensor.transpose(
                tks[:, hp], kt_[hp][:, tt * 128 : (tt + 1) * 128], g.ident
            )
        kn = g.kvp.tile([128, FC], BF16, tag=f"kn{tt}", name=f"kn{tt}")
        nc.scalar.activation(
            kn[:].rearrange("p (a b) -> p a b", a=HP), tks, AF.Copy
        )
        knat.append(kn)

        tvs = g.ps_qkv.tile([128, HP, 2, D], BF16, tag="qkv", name="tvs")
        for hp in range(HP):
            nc.tensor.transpose(
                tvs[:, hp].rearrange("p a b -> p (a b)"),
                vt[hp][:, tt * 128 : (tt + 1) * 128],
                g.ident,
            )
        va = g.kvp.tile([128, HG, D + 1], BF16, tag=f"va{tt}", name=f"va{tt}")
        nc.scalar.activation(
            va[:, :, 0:D], tvs[:].rearrange("p a b c -> p (a b) c"), AF.Copy
        )
        nc.vector.memset(va[:, :, D], 1.0)
        vaug.append(va)
    return knat, vaug


def _chunk(g: _Ctx, c, hp, qt, kt_, knat, vaug, yts):
    """One (chunk, head-pair) step of the chunked linear attention."""
    nc = g.nc
    q0 = c * S
    tt0, tt1 = 2 * c, 2 * c + 1
    qth, kth = qt[hp], kt_[hp]
    # working bf16 copy of the f32 state for the PE
    kvb = g.ynp.tile([128, D + 1], BF16, tag="kvb", name="kvb")
    nc.scalar.activation(kvb, g.kv[hp], AF.Copy)
    # Per head: scores P [s_k-tile(2), s_q] in its own PSUM bank (two
    # matmuls share a row group, so sequential same-bank writes are safe;
    # across heads the row groups differ, hence separate banks).
    pms = []
    for h in range(2):
        r0, r1 = h * D, (h + 1) * D
        ph = g.pp[h]
        nc.tensor.matmul(
            ph[:, 0],
            kth[r0:r1, q0 : q0 + 128],
            qth[r0:r1, q0 : q0 + S],
            tile_position=(h * D, 0),
        )
        # k-tile 1 only scores the causally valid right half of s_q.
        nc.tensor.matmul(
            ph[:, 1, 128:S],
            kth[r0:r1, q0 + 128 : q0 + S],
            qth[r0:r1, q0 + 128 : q0 + S],
            tile_position=(h * D, 0),
        )
        pm = g.pmp.tile([128, 2, S], BF16, tag=f"pm{h}", name=f"pm{h}")
        nc.vector.tensor_mul(pm, ph, g.mask)
        pms.append(pm)
    # y natural [s_q, (u, 65)]: per-head PSUM bank
    ys = []
    for h in range(2):
        r0, r1 = h * D, (h + 1) * D
        hv = hp * 2 + h
        v0 = vaug[tt0][:, hv]
        v1 = vaug[tt1][:, hv]
        pm = pms[h]
        y = getattr(g, f"ps_y{h}").tile(
            [128, 2, D + 1], F32, tag=f"y{h}", name=f"y{h}"
        )
        nc.tensor.matmul(y[:, 0], pm[:, 0, 0:128], v0, start=True, stop=False)
        nc.tensor.matmul(
            y[:, 0],
            qth[r0:r1, q0 : q0 + 128],
            kvb[r0:r1, :],
            start=False, stop=True,
            tile_position=(h * D, 0),
        )
        nc.tensor.matmul(y[:, 1], pm[:, 0, 128:S], v0, start=True, stop=False)
        nc.tensor.matmul(y[:, 1], pm[:, 1, 128:S], v1, start=False, stop=False)
        nc.tensor.matmul(
            y[:, 1],
            qth[r0:r1, q0 + 128 : q0 + S],
            kvb[r0:r1, :],
            start=False, stop=True,
            tile_position=(h * D, 0),
        )
        ys.append(y)
    # state update: KV += Knat^T @ V_aug (heads col-tiled; one bank is fine
    # since col groups write disjoint partition ranges)
    dlt = g.ps_d.tile([128, D + 1], F32, tag="d", name="dlt")
    for h in range(2):
        hv = hp * 2 + h
        nc.tensor.matmul(
            dlt[h * D : (h + 1) * D, :],
            knat[tt0][:, hv * D : (hv + 1) * D],
            vaug[tt0][:, hv],
            start=True, stop=False,
            tile_position=(0, h * D),
        )
        nc.tensor.matmul(
            dlt[h * D : (h + 1) * D, :],
            knat[tt1][:, hv * D : (hv + 1) * D],
            vaug[tt1][:, hv],
            start=False, stop=True,
            tile_position=(0, h * D),
        )
    # normalize y_n = y[..., 0:64] / y[..., 64], transpose into yts
    for h in range(2):
        y = ys[h]
        rec = g.ynp.tile([128, 2], F32, tag="rec", name="rec")
        nc.vector.reciprocal(rec, y[:, :, D])
        yn = g.ynp.tile([128, 2, D], BF16, tag="yn", name="yn")
        nc.vector.tensor_mul(
            yn, y[:, :, 0:D], rec[:, :, None].to_broadcast((128, 2, D))
        )
        tp = g.ps_t.tile([D, 2, 128], BF16, tag="tp", name="tp")
        for u in range(2):
            nc.tensor.transpose(tp[:, u], yn[:, u], g.ident)
        nc.scalar.activation(
            yts[hp][h * D : (h + 1) * D, q0 : q0 + S],
            tp[:].rearrange("p a b -> p (a b)"),
            AF.Copy,
        )
    # state add (scheduled after the kvb copy above)
    nc.vector.tensor_add(g.kv[hp], g.kv[hp], dlt)


def _proj(g: _Ctx, s, yts, out):
    nc = g.nc
    t0 = s * SLAB
    for tt in range(SLAB // 128):
        for ec in range(2):
            ps = g.ps_qkv.tile([128, SLAB], F32, tag="qkv", name="projps")
            for fp in range(HP):
                nc.tensor.matmul(
                    ps,
                    yts[fp][:, tt * 128 : (tt + 1) * 128],
                    g.wp_sb[:, fp, ec * 512 : (ec + 1) * 512],
                    start=(fp == 0),
                    stop=(fp == HP - 1),
                )
            osb = g.outp.tile([128, 512], BF16, tag="osb", name="osb")
            nc.scalar.activation(osb, ps, AF.Copy)
            nc.sync.dma_start(
                out[t0 + tt * 128 : t0 + (tt + 1) * 128, ec * 512 : (ec + 1) * 512],
                osb,
            )


def build_nc(repeat: int = 1) -> bass.Bass:
    nc = bass.Bass()
    dram = {
        "xt": nc.dram_tensor("xt", [C, T], BF16, kind="ExternalInput"),
        "wq": nc.dram_tensor("wq", [C, FC], BF16, kind="ExternalInput"),
        "wk": nc.dram_tensor("wk", [C, FC], BF16, kind="ExternalInput"),
        "wv": nc.dram_tensor("wv", [C, FC], BF16, kind="ExternalInput"),
        "bias": nc.dram_tensor("bias", [128, 5, HP], F32, kind="ExternalInput"),
        "wp": nc.dram_tensor("wp", [FC, C], BF16, kind="ExternalInput"),
    }
    out = nc.dram_tensor("out", [T, C], BF16, kind="ExternalOutput")

    with ExitStack() as ctx:
        tc = ctx.enter_context(tile.TileContext(nc))
        g = _Ctx()
        g.nc = nc
        for nm, kw in (
            ("consts", dict(bufs=1)),
            ("xtp", dict(bufs=1)),
            ("qkp", dict(bufs=3)),
            ("kvp", dict(bufs=2)),
            ("ytp", dict(bufs=2)),
            ("ep", dict(bufs=4)),
            ("pmp", dict(bufs=3)),
            ("ynp", dict(bufs=4)),
            ("outp", dict(bufs=3)),
            ("statep", dict(bufs=1)),
            ("ps_qkv", dict(bufs=2, space="PSUM")),
            ("ps_p0", dict(bufs=1, space="PSUM")),
            ("ps_p1", dict(bufs=1, space="PSUM")),
            ("ps_y0", dict(bufs=1, space="PSUM")),
            ("ps_y1", dict(bufs=1, space="PSUM")),
            ("ps_d", dict(bufs=1, space="PSUM")),
            ("ps_t", dict(bufs=1, space="PSUM")),
        ):
            setattr(g, nm, ctx.enter_context(tc.tile_pool(name=nm, **kw)))

        _setup_consts(g, dram)

        for s in range(NSLAB):
            qt, kt_, vt = _feature_major(g, s)
            knat, vaug = _naturalize(g, kt_, vt)
            yts = [
                g.ytp.tile([128, SLAB], BF16, tag=f"yt{hp}", name=f"yt{hp}")
                for hp in range(HP)
            ]
            for c in range(2):
                for hp in range(HP):
                    _chunk(g, c, hp, qt, kt_, knat, vaug, yts)
            _proj(g, s, yts, out)
    _split_waits(nc)
    return nc


_NC_CACHE = None


def _get_nc():
    global _NC_CACHE
    if _NC_CACHE is None:
        _NC_CACHE = build_nc()
    return _NC_CACHE


def make_in_maps(x, W_attn, b_attn, W_proj):
    x = np.asarray(x, dtype=np.float32)
    W_attn = np.asarray(W_attn, dtype=np.float32)
    b_attn = np.asarray(b_attn, dtype=np.float32)
    W_proj = np.asarray(W_proj, dtype=np.float32)
    bf = ml_dtypes.bfloat16
    in_maps = []
    for core in range(N_CORES):
        b, gg = core // 2, core % 2
        qs, ks, vs = gg * FC, C + gg * FC, 2 * C + gg * FC
        bqc = b_attn[qs : qs + FC].reshape(HP, 128).T
        bkc = b_attn[ks : ks + FC].reshape(HP, 128).T
        bvc = b_attn[vs : vs + FC].reshape(HP, 128).T
        bias = np.stack([bqc, bqc + 1.0, bkc, bkc + 1.0, bvc], axis=1)
        in_maps.append({
            "xt": np.ascontiguousarray(x[b].T).astype(bf),
            "wq": np.ascontiguousarray(W_attn[:, qs : qs + FC]).astype(bf),
            "wk": np.ascontiguousarray(W_attn[:, ks : ks + FC]).astype(bf),
            "wv": np.ascontiguousarray(W_attn[:, vs : vs + FC]).astype(bf),
            "bias": np.ascontiguousarray(bias),
            "wp": np.ascontiguousarray(W_proj[gg * FC : (gg + 1) * FC, :]).astype(bf),
        })
    return in_maps


def kernel(x, W_attn, b_attn, W_proj, b_proj, _trace=False):
    nc = _get_nc()
    in_maps = make_in_maps(x, W_attn, b_attn, W_proj)
    try:
        res = run_bass_kernel_spmd(
            nc, in_maps, core_ids=list(range(N_CORES)), trace=_trace
        )
    except ModuleNotFoundError:
        # axon NTFF profiling hook unavailable in this environment
        res = run_bass_kernel_spmd(
            nc, in_maps, core_ids=list(range(N_CORES)), trace=False
        )
    b_proj = np.asarray(b_proj, dtype=np.float32)
    parts = [r["out"].astype(np.float32) for r in res.results]
    out = np.stack(
        [parts[2 * b] + parts[2 * b + 1] + b_proj for b in range(B)]
    ).astype(np.float32)
    kernel.last_results = res
    return out


# revision 12
# speedup vs baseline: 1.4638x; 1.1901x over previous
"""Trainium2 Bass kernel for linear causal self-attention (ELU+1 feature map).

Model (per batch b):
    qkv = x @ W_attn + b_attn ; q,k,v split; per-head (H=16, d=64)
    phi = elu(.)+1 applied to q,k
    causal linear attention: y_t = (phi_q_t . KV_t) / (phi_q_t . Ksum_t + eps)
        KV_t = sum_{s<=t} phi_k_s (x) v_s ; Ksum_t = sum_{s<=t} phi_k_s
    out = y @ W_proj + b_proj

Sharding (8 cores): core = 2*b + g  (b in 0..3 batches, g in 0..1 head-groups
of 8 heads).  Each core computes a partial output for its batch over its 8
heads; the host sums the two partials per batch and adds b_proj.

On-chip: chunked linear attention (chunk S=256).  All matmuls run in bf16
(f32 PSUM accumulation; the KV state master, denominators and normalization
stay f32).  q/k are produced feature-major; v is produced token-major
directly (x-tile stationary); phi(k) is re-laid-out token-major via PE
transposes.  ELU+1 is computed exactly as max(min(exp(x+b), 1), (x+b)+1)
with one ScalarE Exp pass and one fused custom DVE op.
"""

from contextlib import ExitStack

import ml_dtypes
import numpy as np

import concourse.bass as bass
import concourse.mybir as mybir
import concourse.tile as tile
from concourse.bass_utils import run_bass_kernel_spmd
from concourse.masks import make_identity

F32 = mybir.dt.float32
BF16 = mybir.dt.bfloat16
AF = mybir.ActivationFunctionType

# Problem shape (hardcoded per harness contract)
B, T, C = 4, 2048, 1024
NH, D = 16, 64          # heads total, head dim
HG = 8                  # heads per core (head-group)
HP = 4                  # head-pairs per core (2 heads stacked on 128 partitions)
FC = HG * D             # 512 features per core per q/k/v
KT = C // 128           # 8 k-tiles of the model dim
SLAB = 512              # tokens per slab
NSLAB = T // SLAB       # 4
S = 256                 # chunk length (2 chunks per slab)
N_CORES = 8


def _split_waits(nc, max_waits=1):
    """This walrus build accepts at most one embedded sync-wait per
    instruction; hoist extras into standalone EventSemaphore instructions."""
    n = 0
    for fn in nc.m.functions:
        for bb in fn.blocks:
            new_insts = []
            for inst in bb.instructions:
                si = inst.sync_info
                if si is not None and si.on_wait and len(si.on_wait) > max_waits:
                    extra = si.on_wait[:-max_waits]
                    keep = si.on_wait[-max_waits:]
                    for w in extra:
                        ev = mybir.InstEventSemaphore(
                            name=f"{inst.name}-wsplit{n}",
                            ins=[], outs=[],
                            engine=inst.engine,
                            sync_info=mybir.SyncInfo(on_wait=[w], on_update=[]),
                        )
                        n += 1
                        new_insts.append(ev)
                    si.on_wait = list(keep)
                new_insts.append(inst)
            bb.instructions = new_insts
    return n


class _Ctx:
    """Holds nc, pools and per-kernel constant tiles."""


def _setup_consts(g: _Ctx, dram):
    """Emit const DMAs in critical-path order: wq + biases + xt slab 0
    first (q compute for slab 0 needs exactly these), then wk/wv, the
    remaining x slabs, and wp (only needed ~60us in at first proj)."""
    nc, consts = g.nc, g.consts
    xt_r = dram["xt"][:].rearrange("(kt p) t -> p kt t", p=128)

    g.wq_sb = consts.tile([128, KT, FC], BF16)
    nc.sync.dma_start(g.wq_sb, dram["wq"][:].rearrange("(kt p) f -> p kt f", p=128))
    # combined scalar biases: [128, 5, HP] = (bq, bq1, bk, bk1, bv)
    g.bias_sb = consts.tile([128, 5, HP], F32)
    nc.sync.dma_start(g.bias_sb, dram["bias"][:])
    g.bq_sb = g.bias_sb[:, 0]
    g.bq1_sb = g.bias_sb[:, 1]
    g.bk_sb = g.bias_sb[:, 2]
    g.bk1_sb = g.bias_sb[:, 3]
    g.bv_sb = g.bias_sb[:, 4]

    g.xt_sb = []
    for s in range(NSLAB):
        xt = g.xtp.tile([128, KT, SLAB], BF16, tag=f"xt{s}", name=f"xt{s}")
        g.xt_sb.append(xt)
    nc.sync.dma_start(g.xt_sb[0], xt_r[:, :, 0:SLAB])

    g.wk_sb = consts.tile([128, KT, FC], BF16)
    nc.sync.dma_start(g.wk_sb, dram["wk"][:].rearrange("(kt p) f -> p kt f", p=128))
    g.wv_sb = consts.tile([128, KT, FC], BF16)
    nc.sync.dma_start(g.wv_sb, dram["wv"][:].rearrange("(kt p) f -> p kt f", p=128))

    for s in range(1, NSLAB):
        nc.sync.dma_start(g.xt_sb[s], xt_r[:, :, s * SLAB : (s + 1) * SLAB])

    g.wp_sb = consts.tile([128, HP, C], BF16)
    nc.sync.dma_start(g.wp_sb, dram["wp"][:].rearrange("(fp p) e -> p fp e", p=128))

    # combined causal mask for a [s_k-tile(2), s_q(256)] score block:
    # slot 0 (k-tile 0): keep s_q >= s_k  (tri | all-ones)
    # slot 1 (k-tile 1): left half 0, right half tri
    g.mask = consts.tile([128, 2, S], F32)
    nc.vector.memset(g.mask, 1.0)
    nc.gpsimd.affine_select(
        out=g.mask[:, 0], in_=g.mask[:, 0], compare_op=mybir.AluOpType.is_ge,
        fill=0.0, base=0, pattern=[[1, S]], channel_multiplier=-1,
    )
    nc.gpsimd.affine_select(
        out=g.mask[:, 1], in_=g.mask[:, 1], compare_op=mybir.AluOpType.is_ge,
        fill=0.0, base=-128, pattern=[[1, S]], channel_multiplier=-1,
    )
    g.ident = consts.tile([128, 128], BF16)
    make_identity(nc, g.ident)

    # per-head-pair state master (f32): rows 0:64 head A, 64:128 head B;
    # cols 0:64 = KV[i, j], col 64 = Ksum[i]
    g.kv = []
    for hp in range(HP):
        st = g.statep.tile([128, D + 1], F32, tag=f"kv{hp}", name=f"kv{hp}")
        nc.vector.memset(st, 0.0)
        g.kv.append(st)

    # score PSUM tiles (hoisted; bufs=1 pools).  The masked k-tile-1 matmul
    # only writes the right half; zero the never-written left half once so
    # the mask multiply never reads NaN garbage.
    g.pp = []
    for h in range(2):
        ph = getattr(g, f"ps_p{h}").tile([128, 2, S], F32, tag=f"p{h}", name=f"p{h}")
        nc.vector.memset(ph[:, 1, 0:128], 0.0)
        g.pp.append(ph)


def _fm_group(g: _Ctx, s, which, hp, hold):
    """One q/k/v feature-major PSUM group; ELU+1 on q,k; bias on v."""
    nc = g.nc
    xt_sb = g.xt_sb[s]
    wsb = {"q": g.wq_sb, "k": g.wk_sb, "v": g.wv_sb}[which]
    ps = g.ps_qkv.tile([128, SLAB], F32, tag="qkv", name="qkvps")
    for k in range(KT):
        nc.tensor.matmul(
            ps,
            wsb[:, k, hp * 128 : (hp + 1) * 128],
            xt_sb[:, k, :],
            start=(k == 0),
            stop=(k == KT - 1),
        )
    dst = g.qkp.tile([128, SLAB], BF16, tag=f"{which}t{hp}", name=f"{which}t{hp}")
    if which == "v":
        nc.scalar.activation(
            dst, ps, AF.Identity, bias=g.bv_sb[:, hp : hp + 1], scale=1.0
        )
    else:
        # elu(y)+1 = max(min(exp(y), 1), y+1) with y = x + b
        bsb = g.bq_sb if which == "q" else g.bk_sb
        b1sb = g.bq1_sb if which == "q" else g.bk1_sb
        e = g.ep.tile([128, SLAB], BF16, tag="e", name="e")
        nc.scalar.activation(
            e, ps, AF.Exp, bias=bsb[:, hp : hp + 1], scale=1.0
        )
        nc.vector.tensor_scalar_min(e, e, 1.0)
        nc.vector.scalar_tensor_tensor(
            dst, ps, b1sb[:, hp : hp + 1], e,
            mybir.AluOpType.add, mybir.AluOpType.max,
        )
    hold[which].append(dst)


def _nat_tt(g: _Ctx, tt, hold):
    """PE-transpose phi(k) and v into [token, feature] tiles for one
    128-token block; v gets the ones column (col 64 per head) used to
    carry Ksum / the denominator."""
    nc = g.nc
    kt_, vt = hold["k"], hold["v"]
    tks = g.ps_qkv.tile([128, HP, 128], BF16, tag="qkv", name="tks")
    for hp in range(HP):
        nc.tensor.transpose(
            tks[:, hp], kt_[hp][:, tt * 128 : (tt + 1) * 128], g.ident
        )
    kn = g.kvp.tile([128, FC], BF16, tag=f"kn{tt}", name=f"kn{tt}")
    nc.scalar.activation(
        kn[:].rearrange("p (a b) -> p a b", a=HP), tks, AF.Copy
    )
    hold["knat"].append(kn)

    tvs = g.ps_qkv.tile([128, HP, 2, D], BF16, tag="qkv", name="tvs")
    for hp in range(HP):
        nc.tensor.transpose(
            tvs[:, hp].rearrange("p a b -> p (a b)"),
            vt[hp][:, tt * 128 : (tt + 1) * 128],
            g.ident,
        )
    va = g.kvp.tile([128, HG, D + 1], BF16, tag=f"va{tt}", name=f"va{tt}")
    nc.scalar.activation(
        va[:, :, 0:D], tvs[:].rearrange("p a b c -> p (a b) c"), AF.Copy
    )
    nc.vector.memset(va[:, :, D], 1.0)
    hold["vaug"].append(va)


def _slab_B_units(g: _Ctx, s):
    """Emission units producing slab s's q/k/v + naturalized tiles:
    12 dense GEMM groups + 4 transpose blocks."""
    hold = {"q": [], "k": [], "v": [], "knat": [], "vaug": []}
    units = []
    for which in ("q", "k", "v"):
        for hp in range(HP):
            units.append(lambda which=which, hp=hp: _fm_group(g, s, which, hp, hold))
    for tt in range(SLAB // 128):
        units.append(lambda tt=tt: _nat_tt(g, tt, hold))
    return units, hold


def _slab_A_units(g: _Ctx, s, hold, out):
    """Emission units consuming slab s's tiles: 8 chunk calls split in
    two (scores / y+state) + 8 proj groups."""
    yts = [
        g.ytp.tile([128, SLAB], BF16, tag=f"yt{hp}", name=f"yt{hp}")
        for hp in range(HP)
    ]
    units = []
    for c in range(2):
        for hp in range(HP):
            slot = {}

            def ua(c=c, hp=hp, slot=slot):
                slot["kvb"], slot["pms"] = _chunk_a(g, c, hp, hold["q"], hold["k"])

            def ub(c=c, hp=hp, slot=slot):
                _chunk_b(
                    g, c, hp, hold["q"], hold["knat"], hold["vaug"],
                    yts, slot["kvb"], slot["pms"],
                )

            units.append(ua)
            units.append(ub)
    for tt in range(SLAB // 128):
        for ec in range(2):
            units.append(lambda tt=tt, ec=ec: _proj_group(g, s, yts, out, tt, ec))
    return units


def _weave(a_units, b_units):
    """Merge two unit lists by fractional position (stable)."""
    merged = []
    ia = ib = 0
    na, nb = len(a_units), len(b_units)
    while ia < na or ib < nb:
        if ib >= nb or (ia < na and ia * (nb or 1) <= ib * na):
            merged.append(a_units[ia]); ia += 1
        else:
            merged.append(b_units[ib]); ib += 1
    return merged


def _chunk_a(g: _Ctx, c, hp, qt, kt_):
    """Scores + state snapshot for one (chunk, head-pair)."""
    nc = g.nc
    q0 = c * S
    qth, kth = qt[hp], kt_[hp]
    # working bf16 copy of the f32 state for the PE
    kvb = g.ynp.tile([128, D + 1], BF16, tag="kvb", name="kvb")
    nc.scalar.activation(kvb, g.kv[hp], AF.Copy)
    # Per head: scores P [s_k-tile(2), s_q] in its own PSUM bank (two
    # matmuls share a row group, so sequential same-bank writes are safe;
    # across heads the row groups differ, hence separate banks).
    pms = []
    for h in range(2):
        r0, r1 = h * D, (h + 1) * D
        ph = g.pp[h]
        nc.tensor.matmul(
            ph[:, 0],
            kth[r0:r1, q0 : q0 + 128],
            qth[r0:r1, q0 : q0 + S],
            tile_position=(h * D, 0),
        )
        # k-tile 1 only scores the causally valid right half of s_q.
        nc.tensor.matmul(
            ph[:, 1, 128:S],
            kth[r0:r1, q0 + 128 : q0 + S],
            qth[r0:r1, q0 + 128 : q0 + S],
            tile_position=(h * D, 0),
        )
        pm = g.pmp.tile([128, 2, S], BF16, tag=f"pm{h}", name=f"pm{h}")
        nc.vector.tensor_mul(pm, ph, g.mask)
        pms.append(pm)
    return kvb, pms


def _chunk_b(g: _Ctx, c, hp, qt, knat, vaug, yts, kvb, pms):
    """y accumulation, state update and normalization for one
    (chunk, head-pair)."""
    nc = g.nc
    q0 = c * S
    tt0, tt1 = 2 * c, 2 * c + 1
    qth = qt[hp]
    # y natural [s_q, (u, 65)]: per-head PSUM bank
    ys = []
    for h in range(2):
        r0, r1 = h * D, (h + 1) * D
        hv = hp * 2 + h
        v0 = vaug[tt0][:, hv]
        v1 = vaug[tt1][:, hv]
        pm = pms[h]
        y = getattr(g, f"ps_y{h}").tile(
            [128, 2, D + 1], F32, tag=f"y{h}", name=f"y{h}"
        )
        nc.tensor.matmul(y[:, 0], pm[:, 0, 0:128], v0, start=True, stop=False)
        nc.tensor.matmul(
            y[:, 0],
            qth[r0:r1, q0 : q0 + 128],
            kvb[r0:r1, :],
            start=False, stop=True,
            tile_position=(h * D, 0),
        )
        nc.tensor.matmul(y[:, 1], pm[:, 0, 128:S], v0, start=True, stop=False)
        nc.tensor.matmul(y[:, 1], pm[:, 1, 128:S], v1, start=False, stop=False)
        nc.tensor.matmul(
            y[:, 1],
            qth[r0:r1, q0 + 128 : q0 + S],
            kvb[r0:r1, :],
            start=False, stop=True,
            tile_position=(h * D, 0),
        )
        ys.append(y)
    # state update: KV += Knat^T @ V_aug (heads col-tiled; one bank is fine
    # since col groups write disjoint partition ranges)
    dlt = g.ps_d.tile([128, D + 1], F32, tag="d", name="dlt")
    for h in range(2):
        hv = hp * 2 + h
        nc.tensor.matmul(
            dlt[h * D : (h + 1) * D, :],
            knat[tt0][:, hv * D : (hv + 1) * D],
            vaug[tt0][:, hv],
            start=True, stop=False,
            tile_position=(0, h * D),
        )
        nc.tensor.matmul(
            dlt[h * D : (h + 1) * D, :],
            knat[tt1][:, hv * D : (hv + 1) * D],
            vaug[tt1][:, hv],
            start=False, stop=True,
            tile_position=(0, h * D),
        )
    # normalize y_n = y[..., 0:64] / y[..., 64], transpose into yts
    for h in range(2):
        y = ys[h]
        rec = g.ynp.tile([128, 2], F32, tag="rec", name="rec")
        nc.vector.reciprocal(rec, y[:, :, D])
        yn = g.ynp.tile([128, 2, D], BF16, tag="yn", name="yn")
        nc.vector.tensor_mul(
            yn, y[:, :, 0:D], rec[:, :, None].to_broadcast((128, 2, D))
        )
        tp = g.ps_t.tile([D, 2, 128], BF16, tag="tp", name="tp")
        for u in range(2):
            nc.tensor.transpose(tp[:, u], yn[:, u], g.ident)
        nc.scalar.activation(
            yts[hp][h * D : (h + 1) * D, q0 : q0 + S],
            tp[:].rearrange("p a b -> p (a b)"),
            AF.Copy,
        )
    # state add (scheduled after the kvb copy above)
    nc.vector.tensor_add(g.kv[hp], g.kv[hp], dlt)


def _proj_group(g: _Ctx, s, yts, out, tt, ec):
    nc = g.nc
    t0 = s * SLAB
    ps = g.ps_qkv.tile([128, SLAB], F32, tag="qkv", name="projps")
    for fp in range(HP):
        nc.tensor.matmul(
            ps,
            yts[fp][:, tt * 128 : (tt + 1) * 128],
            g.wp_sb[:, fp, ec * 512 : (ec + 1) * 512],
            start=(fp == 0),
            stop=(fp == HP - 1),
        )
    osb = g.outp.tile([128, 512], BF16, tag="osb", name="osb")
    nc.scalar.activation(osb, ps, AF.Copy)
    nc.sync.dma_start(
        out[t0 + tt * 128 : t0 + (tt + 1) * 128, ec * 512 : (ec + 1) * 512],
        osb,
    )


def build_nc(repeat: int = 1) -> bass.Bass:
    nc = bass.Bass()
    dram = {
        "xt": nc.dram_tensor("xt", [C, T], BF16, kind="ExternalInput"),
        "wq": nc.dram_tensor("wq", [C, FC], BF16, kind="ExternalInput"),
        "wk": nc.dram_tensor("wk", [C, FC], BF16, kind="ExternalInput"),
        "wv": nc.dram_tensor("wv", [C, FC], BF16, kind="ExternalInput"),
        "bias": nc.dram_tensor("bias", [128, 5, HP], F32, kind="ExternalInput"),
        "wp": nc.dram_tensor("wp", [FC, C], BF16, kind="ExternalInput"),
    }
    out = nc.dram_tensor("out", [T, C], BF16, kind="ExternalOutput")

    with ExitStack() as ctx:
        tc = ctx.enter_context(tile.TileContext(nc))
        g = _Ctx()
        g.nc = nc
        for nm, kw in (
            ("consts", dict(bufs=1)),
            ("xtp", dict(bufs=1)),
            ("qkp", dict(bufs=3)),
            ("kvp", dict(bufs=2)),
            ("ytp", dict(bufs=2)),
            ("ep", dict(bufs=4)),
            ("pmp", dict(bufs=3)),
            ("ynp", dict(bufs=4)),
            ("outp", dict(bufs=3)),
            ("statep", dict(bufs=1)),
            ("ps_qkv", dict(bufs=2, space="PSUM")),
            ("ps_p0", dict(bufs=1, space="PSUM")),
            ("ps_p1", dict(bufs=1, space="PSUM")),
            ("ps_y0", dict(bufs=1, space="PSUM")),
            ("ps_y1", dict(bufs=1, space="PSUM")),
            ("ps_d", dict(bufs=1, space="PSUM")),
            ("ps_t", dict(bufs=1, space="PSUM")),
        ):
            setattr(g, nm, ctx.enter_context(tc.tile_pool(name=nm, **kw)))

        _setup_consts(g, dram)

        # software pipeline: slab s's attention/proj (A) woven with slab
        # s+1's dense qkv GEMM + transposes (B).  Keeps dense matmuls in
        # the PE mix at all times (flattens power draw, hides the
        # DVE/ACT round-trips of the chunk recurrences).
        b_units, hold = _slab_B_units(g, 0)
        for u in b_units:
            u()
        for s in range(NSLAB):
            a_units = _slab_A_units(g, s, hold, out)
            if s + 1 < NSLAB:
                b_units, hold = _slab_B_units(g, s + 1)
            else:
                b_units = []
            for u in _weave(a_units, b_units):
                u()
    _split_waits(nc)
    return nc


_NC_CACHE = None


def _get_nc():
    global _NC_CACHE
    if _NC_CACHE is None:
        _NC_CACHE = build_nc()
    return _NC_CACHE


def make_in_maps(x, W_attn, b_attn, W_proj):
    x = np.asarray(x, dtype=np.float32)
    W_attn = np.asarray(W_attn, dtype=np.float32)
    b_attn = np.asarray(b_attn, dtype=np.float32)
    W_proj = np.asarray(W_proj, dtype=np.float32)
    bf = ml_dtypes.bfloat16
    in_maps = []
    for core in range(N_CORES):
        b, gg = core // 2, core % 2
        qs, ks, vs = gg * FC, C + gg * FC, 2 * C + gg * FC
        bqc = b_attn[qs : qs + FC].reshape(HP, 128).T
        bkc = b_attn[ks : ks + FC].reshape(HP, 128).T
        bvc = b_attn[vs : vs + FC].reshape(HP, 128).T
        bias = np.stack([bqc, bqc + 1.0, bkc, bkc + 1.0, bvc], axis=1)
        in_maps.append({
            "xt": np.ascontiguousarray(x[b].T).astype(bf),
            "wq": np.ascontiguousarray(W_attn[:, qs : qs + FC]).astype(bf),
            "wk": np.ascontiguousarray(W_attn[:, ks : ks + FC]).astype(bf),
            "wv": np.ascontiguousarray(W_attn[:, vs : vs + FC]).astype(bf),
            "bias": np.ascontiguousarray(bias),
            "wp": np.ascontiguousarray(W_proj[gg * FC : (gg + 1) * FC, :]).astype(bf),
        })
    return in_maps


def kernel(x, W_attn, b_attn, W_proj, b_proj, _trace=False):
    nc = _get_nc()
    in_maps = make_in_maps(x, W_attn, b_attn, W_proj)
    try:
        res = run_bass_kernel_spmd(
            nc, in_maps, core_ids=list(range(N_CORES)), trace=_trace
        )
    except ModuleNotFoundError:
        # axon NTFF profiling hook unavailable in this environment
        res = run_bass_kernel_spmd(
            nc, in_maps, core_ids=list(range(N_CORES)), trace=False
        )
    b_proj = np.asarray(b_proj, dtype=np.float32)
    parts = [r["out"].astype(np.float32) for r in res.results]
    out = np.stack(
        [parts[2 * b] + parts[2 * b + 1] + b_proj for b in range(B)]
    ).astype(np.float32)
    kernel.last_results = res
    return out
